# revision 9
# baseline (speedup 1.0000x reference)
"""Trainium2 Bass kernel for nn_AttentionBlock_68624987455817.

Pre-LN causal self-attention block + MLP (B=8, L=1024, E=768, H=12, D=64).

Sharding: data-parallel over batch B=8 across the 8 NeuronCores (one batch
element per core, weights replicated, no collectives). Each core runs the
full block on its [1024, 768] slice.

v2 schedule (from the 323us baseline, targeting the PE/ACT co-roofline):
  - prologue: x tile DMAs split/staged, PE warmed with junk matmuls so HAM
    unthrottles before real work; LN rsqrt via DVE bit-trick (no ACT table),
    single Exp table load for the whole kernel.
  - ph0: per-tile LN1 (DVE stats/magic-rsqrt/apply) -> PE transpose -> V
    matmuls; psum->SBUF copies on ACT; qk(0) column halves emitted as their
    z1T dependencies complete.
  - ph3: per head pair, S^T / exp pipelined through a 4-deep single-bank
    psum ring with qk(c+1) matmuls interleaved between key tiles so the PE
    covers the ACT exp drain; P^T stored as one [P, LT, L] tile per parity
    (one exp per (kt,seg)); PV normalization multiplies straight out of
    PSUM (recip+partition-broadcast) into OT.
  - ph4/5/6 merged: proj -> x1 -> LN2 (DVE) -> z2T transposes run
    interleaved with the fc matmul stream (fc lc=0 starts once tiles 0-3
    are transposed); selu = ACT exp + ACT relu + one DVE combine, with the
    -lambda*alpha constant folded into an effective output bias host-side;
    out as two column passes sharing the fc psum ring.
  - x / x1 kept bf16 for the merged phase (f32 staging for LN1 stats);
    weights bf16 (rel err ~3e-3 vs the 2e-2 gate).
"""
import os
import sys

sys.path.insert(0, "/opt/trn_rl_repo")

import numpy as np
import ml_dtypes

import concourse.bass as bass
from concourse import bacc
import concourse.mybir as mybir
from concourse.tile import TileContext
from concourse import bass_utils
from concourse.masks import make_identity

F32 = mybir.dt.float32
F32R = mybir.dt.float32r
BF16 = mybir.dt.bfloat16
I32 = mybir.dt.int32
AF = mybir.ActivationFunctionType
OP = mybir.AluOpType
AX = mybir.AxisListType

P = 128
L = 1024
E = 768
H = 12
D = 64
DA = D + 1           # V columns + ones column (row-sum trick)
EC = E // P          # 6 feature chunks
LT = L // P          # 8 token tiles
QC = L // 512        # 2 query chunks
KC2 = 4 * E // P     # 24 chunks of the MLP hidden dim
NCORES = 8

SELU_LAMBDA = 1.0507009873554805
SELU_ALPHA = 1.6732632423543772
SELU_LA = SELU_LAMBDA * SELU_ALPHA
LN_EPS = 1e-6
MAGIC = 0x5F3759DF

_last_results = None
_build_cache = {}


def _build(gates, mm_dt_name):
    MDT = {"f32r": F32R, "bf16": BF16}[mm_dt_name]
    PSDT = MDT  # transpose output dtype must match its input dtype

    nc = bacc.Bacc("TRN2", target_bir_lowering=False)

    x_d = nc.dram_tensor("x", [L, E], F32, kind="ExternalInput")
    wqk_d = nc.dram_tensor("wqk", [E, 2 * E], MDT, kind="ExternalInput")
    wv_d = nc.dram_tensor("wv", [E, E], MDT, kind="ExternalInput")
    wproj_d = nc.dram_tensor("wproj", [E, E], MDT, kind="ExternalInput")
    wfc_d = nc.dram_tensor("wfc", [E, 4 * E], MDT, kind="ExternalInput")
    wout_d = nc.dram_tensor("wout", [4 * E, E], MDT, kind="ExternalInput")
    out_d = nc.dram_tensor("out", [L, E], F32, kind="ExternalOutput")

    bqk_d = bv_d = bproj_d = bfce_d = bfcl_d = bout_d = None
    if gates["bqk"]:
        bqk_d = nc.dram_tensor("bqk", [2 * E], F32, kind="ExternalInput")
    if gates["bv"]:
        bv_d = nc.dram_tensor("bv", [E], F32, kind="ExternalInput")
    if gates["bproj"]:
        bproj_d = nc.dram_tensor("bproj", [E], F32, kind="ExternalInput")
    if gates["bfc"]:
        bfce_d = nc.dram_tensor("bfce", [4 * E], F32, kind="ExternalInput")
        bfcl_d = nc.dram_tensor("bfcl", [4 * E], F32, kind="ExternalInput")
    if gates["bout"]:
        bout_d = nc.dram_tensor("bout", [E], F32, kind="ExternalInput")

    xv = x_d.rearrange("(t p) e -> p t e", p=P)            # [128, 8, 768]
    wqkv = wqk_d.rearrange("(c p) m -> p c m", p=P)        # [128, 6, 1536]
    wvv = wv_d.rearrange("(c p) m -> p c m", p=P)          # [128, 6, 768]
    wprojv = wproj_d.rearrange("(c p) m -> p c m", p=P)    # [128, 6, 768]
    wfcv = wfc_d.rearrange("(c p) m -> p c m", p=P)        # [128, 6, 3072]
    woutv = wout_d.rearrange("(c p) m -> p c m", p=P)      # [128, 24, 768]
    outv = out_d.rearrange("(t p) e -> p t e", p=P)

    with TileContext(nc) as tc:
        with tc.tile_pool(name="pers", bufs=1) as pers:
            # ---- persistent SBUF ----
            xbf = pers.tile([P, LT, E], BF16)    # x (bf16), live ph0 -> ph4
            x1bf = pers.tile([P, LT, E], BF16)   # x1 (bf16), live ph4 -> ph6

            mask_f32 = pers.tile([P, P], F32)
            ones_f32 = pers.tile([P, LT * H], F32)
            lnla_b = pers.tile([P, 1], F32)
            kmag = pers.tile([P, 1], I32)
            tabs = pers.tile([P, 1], F32)
            ident = pers.tile([P, P], F32)
            ident_m = pers.tile([P, P], MDT)
            if MDT == BF16:
                mask_b = pers.tile([P, P], BF16)

            bqk_sb = bv_sb = bproj_sb = bfce_sb = bfcl_sb = bout_sb = None
            if gates["bqk"]:
                bqk_sb = pers.tile([P, 2 * EC], F32)
            if gates["bv"]:
                bv_sb = pers.tile([P, E], F32)
            if gates["bproj"]:
                bproj_sb = pers.tile([P, E], F32)
            if gates["bfc"]:
                bfce_sb = pers.tile([P, KC2], F32)
                bfcl_sb = pers.tile([P, KC2], F32)
            if gates["bout"]:
                bout_sb = pers.tile([P, E], F32)

            def magic_rsqrt(pool, var_ap, tag):
                """rsqrt(var + eps) on DVE only (bit trick + 2 Newton steps).

                Keeps the ACT table on Exp for the whole kernel."""
                v = pool.tile([P, 1], F32, tag="mg_v", name=f"v{tag}")
                nc.vector.tensor_scalar_add(v[:], var_ap, LN_EPS)
                hb = pool.tile([P, 1], I32, tag="mg_h", name=f"h{tag}")
                nc.vector.tensor_scalar(hb[:], v[:].bitcast(I32), 1, None,
                                        OP.logical_shift_right)
                y = pool.tile([P, 1], F32, tag="mg_y", name=f"y{tag}")
                nc.vector.tensor_tensor(y[:].bitcast(I32), kmag[:], hb[:],
                                        OP.subtract)
                nh = pool.tile([P, 1], F32, tag="mg_n", name=f"n{tag}")
                nc.vector.tensor_scalar(nh[:], v[:], -0.5, None, OP.mult)
                t1 = pool.tile([P, 1], F32, tag="mg_t", name=f"t{tag}")
                for _ in range(2):
                    nc.vector.tensor_tensor(t1[:], y[:], y[:], OP.mult)
                    nc.vector.tensor_scalar(t1[:], t1[:], nh[:], 1.5,
                                            OP.mult, OP.add)
                    nc.vector.tensor_tensor(y[:], y[:], t1[:], OP.mult)
                return y

            def transpose_block(dstT, src_tile, t, pspool):
                """dstT[:, c, t*P:(t+1)*P] = src_tile[:, c*P:(c+1)*P].T"""
                for c in range(EC):
                    pt = pspool.tile([P, P], PSDT, tag="tr")
                    nc.tensor.transpose(pt[:], src_tile[:, c * P:(c + 1) * P],
                                        ident_m[:])
                    nc.scalar.copy(out=dstT[:, c, t * P:(t + 1) * P],
                                   in_=pt[:])

            with tc.tile_pool(name="fm", bufs=1) as fmp:
                z1T = fmp.tile([P, EC, L], MDT, tag="fm", name="z1T")
                with tc.tile_pool(name="wfcp", bufs=1) as wfcp:
                    wfc_sb = wfcp.tile([P, EC, 4 * E], MDT)
                    with (
                        tc.tile_pool(name="otp", bufs=1) as otp,
                        tc.tile_pool(name="wpp", bufs=1) as wpp,
                    ):
                        OT = otp.tile([P, EC, L], MDT)
                        wproj_sb = wpp.tile([P, EC, E], MDT)

                        # ======== ph0 + ph3 super-scope ========
                        with (
                            tc.tile_pool(name="vp", bufs=1) as vpool,
                            tc.tile_pool(name="qkpp", bufs=2) as qkpp,
                            tc.tile_pool(name="wqks", bufs=4) as wqs,
                            tc.tile_pool(name="psqk", bufs=2,
                                         space="PSUM") as ps_qk,
                            tc.tile_pool(name="psa", bufs=2,
                                         space="PSUM") as ps_a,
                        ):
                            v_aug = vpool.tile([P, LT, H, DA], MDT)

                            def emit_qk_dma(c):
                                wts = []
                                for i, oc in enumerate((c, EC + c)):
                                    wt = wqs.tile([P, EC, P], MDT,
                                                  tag=f"wqk{i}",
                                                  name=f"wqk{c}_{i}")
                                    nc.sync.dma_start(
                                        wt[:],
                                        wqkv[:, :, oc * P:(oc + 1) * P])
                                    wts.append(wt)
                                return wts

                            def qk_group(c, wts, qk_pair, i, lcq):
                                """one (q|k, lc) projection group: 6 MMs and
                                the psum->SBUF copy. Generator yields after
                                kc==2 and after the copy."""
                                pq = ps_qk.tile([P, 512], F32, tag="qk")
                                for kc in range(EC):
                                    nc.tensor.matmul(
                                        pq[:], wts[i][:, kc, :],
                                        z1T[:, kc, lcq * 512:(lcq + 1) * 512],
                                        start=(kc == 0), stop=(kc == EC - 1))
                                    if kc == 2:
                                        yield
                                dst = qk_pair[:, i, lcq * 512:(lcq + 1) * 512]
                                if gates["bqk"]:
                                    oc = c if i == 0 else EC + c
                                    nc.scalar.activation(
                                        dst, pq[:], AF.Identity,
                                        bias=bqk_sb[:, oc:oc + 1])
                                else:
                                    nc.vector.tensor_copy(out=dst, in_=pq[:])
                                yield

                            def qk_gen(c, wts, qk_pair):
                                for i in range(2):
                                    for lcq in range(QC):
                                        yield from qk_group(c, wts, qk_pair,
                                                            i, lcq)

                            # ---------------- ph0 ----------------
                            with (
                                tc.tile_pool(name="xfp", bufs=1) as xfp,
                                tc.tile_pool(name="wvp", bufs=1) as wvp,
                                tc.tile_pool(name="zp", bufs=3) as zp,
                                tc.tile_pool(name="stp", bufs=3) as stp,
                                tc.tile_pool(name="ps0", bufs=4,
                                             space="PSUM") as ps0,
                            ):
                                xallf = xfp.tile([P, LT, E], F32)
                                # x0/x1 DMAs issued before any setup; x0 in
                                # halves so LN1 stats start on the first half
                                nc.sync.dma_start(xallf[:, 0, 0:384],
                                                  xv[:, 0, 0:384])
                                nc.sync.dma_start(xallf[:, 0, 384:768],
                                                  xv[:, 0, 384:768])
                                nc.sync.dma_start(xallf[:, 1, :], xv[:, 1, :])

                                wv_sb = wvp.tile([P, EC, E], MDT)
                                nc.sync.dma_start(wv_sb[:, :, 0:512],
                                                  wvv[:, :, 0:512])

                                # ---- constants (overlap the DMAs) ----
                                nc.vector.memset(ones_f32[:], 1.0)
                                nc.vector.memset(lnla_b[:],
                                                 float(np.log(SELU_LA)))
                                nc.vector.memset(kmag[:], MAGIC)
                                nc.vector.memset(tabs[:], 0.5)
                                # single ACT table load for the whole kernel
                                nc.scalar.activation(tabs[:], tabs[:], AF.Exp)
                                nc.gpsimd.memset(mask_f32[:], 0.0)
                                nc.gpsimd.affine_select(
                                    out=mask_f32[:], in_=mask_f32[:],
                                    compare_op=OP.is_ge, fill=1.0, base=-1,
                                    pattern=[[-1, P]], channel_multiplier=1,
                                )
                                if MDT == F32R:
                                    mask_tri = mask_f32[:].bitcast(F32R)
                                else:
                                    nc.vector.tensor_copy(mask_b[:],
                                                          mask_f32[:])
                                    mask_tri = mask_b[:]
                                make_identity(nc, ident)
                                nc.vector.tensor_copy(ident_m[:], ident[:])
                                nc.vector.tensor_copy(
                                    v_aug[:, :, :, D:DA],
                                    ones_f32[:].rearrange(
                                        "p (t h o) -> p t h o", h=H, o=1))

                                # warm the PE (HAM unthrottle) while x lands
                                for _ in range(24):
                                    wps = ps0.tile([P, P], PSDT, tag="tr")
                                    nc.tensor.transpose(wps[:], ident_m[:],
                                                        ident_m[:])

                                # rest of the input DMAs
                                nc.sync.dma_start(wv_sb[:, :, 512:768],
                                                  wvv[:, :, 512:768])
                                for t in range(2, LT):
                                    nc.sync.dma_start(xallf[:, t, :],
                                                      xv[:, t, :])
                                wts0 = emit_qk_dma(0)
                                nc.sync.dma_start(wproj_sb[:], wprojv[:])
                                if gates["bqk"]:
                                    nc.sync.dma_start(
                                        bqk_sb[:],
                                        bqk_d.rearrange("(c p) -> p c", p=P))
                                if gates["bv"]:
                                    nc.sync.dma_start(
                                        bv_sb[:], bv_d.reshape((1, E)).broadcast_to((P, E)))
                                if gates["bproj"]:
                                    nc.sync.dma_start(
                                        bproj_sb[:],
                                        bproj_d.reshape((1, E)).broadcast_to((P, E)))
                                if gates["bfc"]:
                                    nc.sync.dma_start(
                                        bfce_sb[:],
                                        bfce_d.rearrange("(c p) -> p c", p=P))
                                    nc.sync.dma_start(
                                        bfcl_sb[:],
                                        bfcl_d.rearrange("(c p) -> p c", p=P))
                                if gates["bout"]:
                                    nc.sync.dma_start(
                                        bout_sb[:],
                                        bout_d.reshape((1, E)).broadcast_to((P, E)))

                                def ln_stats(t):
                                    bnst = stp.tile([P, 2, 6], F32, tag="bn",
                                                    name=f"bn{t}")
                                    xg = xallf[:, t, :].rearrange(
                                        "p (n f) -> p n f", f=384)
                                    for g in range(2):
                                        nc.vector.bn_stats(bnst[:, g, :],
                                                           xg[:, g, :])
                                    mv = stp.tile([P, 2], F32, tag="mv",
                                                  name=f"mv{t}")
                                    nc.vector.bn_aggr(mv[:], bnst[:])
                                    return mv

                                qkp0 = qkpp.tile([P, 2, L], MDT, tag="qkpair",
                                                 name="qkp0")
                                stats1 = {0: ln_stats(0), 1: ln_stats(1)}
                                for t in range(LT):
                                    mv = stats1.pop(t)
                                    rt = magic_rsqrt(stp, mv[:, 1:2], "1")
                                    zt = zp.tile([P, E], MDT, tag="z")
                                    nc.vector.tensor_scalar(
                                        zt[:], xallf[:, t, :], mv[:, 0:1],
                                        rt[:], OP.subtract, OP.mult)
                                    # bf16 copy of x for the merged phase
                                    nc.scalar.copy(out=xbf[:, t, :],
                                                   in_=xallf[:, t, :])
                                    if t + 2 < LT:
                                        stats1[t + 2] = ln_stats(t + 2)
                                    transpose_block(z1T, zt, t, ps0)
                                    # V matmuls for tile t
                                    for (c0, cw) in ((0, 512), (512, 256)):
                                        pt2 = ps_a.tile([P, 512], F32,
                                                        tag="pv")
                                        for kc in range(EC):
                                            nc.tensor.matmul(
                                                pt2[:, :cw],
                                                z1T[:, kc, t * P:(t + 1) * P],
                                                wv_sb[:, kc, c0:c0 + cw],
                                                start=(kc == 0),
                                                stop=(kc == EC - 1),
                                            )
                                        h0 = c0 // D
                                        nh = cw // D
                                        dst = v_aug[:, t, h0:h0 + nh, 0:D]
                                        src = pt2[:, :cw].rearrange(
                                            "p (h d) -> p h d", d=D)
                                        if gates["bv"]:
                                            nc.vector.tensor_tensor(
                                                dst, src,
                                                bv_sb[:, c0:c0 + cw].rearrange(
                                                    "p (h d) -> p h d", d=D),
                                                OP.add)
                                        else:
                                            nc.scalar.copy(out=dst, in_=src)
                                    # qk(0) lc halves as their z1T tokens
                                    # complete (lc0 needs tiles 0-3)
                                    if t == 4:
                                        for i in range(2):
                                            for _ in qk_group(0, wts0, qkp0,
                                                              i, 0):
                                                pass
                                    if t == LT - 1:
                                        for i in range(2):
                                            for _ in qk_group(0, wts0, qkp0,
                                                              i, 1):
                                                pass

                            # ---------------- ph3 ----------------
                            with (
                                tc.tile_pool(name="ptp", bufs=1) as ptp,
                                tc.tile_pool(name="recp", bufs=2) as recp,
                                tc.tile_pool(name="psst", bufs=4,
                                             space="PSUM") as ps_st,
                            ):
                                def emit_st(qk_pair, PTs, kt):
                                    s0 = kt * P
                                    segs = ([(s0, 512), (512, L)]
                                            if s0 < 512 else [(s0, L)])
                                    for (a, b) in segs:
                                        psegs = []
                                        for par in range(2):
                                            pss = ps_st.tile(
                                                [P, 512], F32, tag="st",
                                                name=f"st{kt}p{par}a{a}")
                                            rows = slice(par * D,
                                                         par * D + D)
                                            nc.tensor.matmul(
                                                pss[:, :b - a],
                                                qk_pair[rows, 1, s0:s0 + P],
                                                qk_pair[rows, 0, a:b],
                                                start=True, stop=True)
                                            psegs.append(pss)
                                        for par in range(2):
                                            nc.scalar.activation(
                                                PTs[par][:, kt, a:b],
                                                psegs[par][:, :b - a], AF.Exp)
                                        if a == s0:
                                            for par in range(2):
                                                nc.vector.tensor_tensor(
                                                    PTs[par][:, kt,
                                                             s0:s0 + P],
                                                    PTs[par][:, kt,
                                                             s0:s0 + P],
                                                    mask_tri, OP.mult)

                                def emit_pv(c, qc, PTs, psos):
                                    q0 = qc * 512
                                    for par in range(2):
                                        h = 2 * c + par
                                        pso = psos[par]
                                        kts = [j for j in range(LT)
                                               if j * P < q0 + 512]
                                        for idx, j in enumerate(kts):
                                            a = max(j * P, q0)
                                            nc.tensor.matmul(
                                                pso[0:DA, a - q0:512],
                                                v_aug[:, j, h, :],
                                                PTs[par][:, j, a:q0 + 512],
                                                start=(idx == 0),
                                                stop=(idx == len(kts) - 1))

                                def emit_norm(c, qc, psos):
                                    # psum row 64 carries the softmax row
                                    # sums; normalize straight out of PSUM.
                                    q0 = qc * 512
                                    for par in range(2):
                                        pso = psos[par]
                                        o_rows = slice(par * D, par * D + D)
                                        srow = recp.tile([P, 512], F32,
                                                         tag="sr")
                                        nc.vector.tensor_copy(srow[0:1, :],
                                                              pso[D:DA, :])
                                        rec = recp.tile([P, 512], F32,
                                                        tag="rc")
                                        nc.vector.reciprocal_approx_fast(
                                            rec[0:1, :], srow[0:1, :])
                                        recb = recp.tile([P, 512], F32,
                                                         tag="rb")
                                        nc.gpsimd.partition_broadcast(
                                            recb[0:D, :], rec[0:1, :])
                                        nc.vector.tensor_tensor(
                                            OT[o_rows, c, q0:q0 + 512],
                                            pso[0:D, :], recb[0:D, :],
                                            OP.mult)

                                qkp_cur = qkp0
                                for c in range(EC):
                                    if c + 1 < EC:
                                        wts_n = emit_qk_dma(c + 1)
                                        qkp_n = qkpp.tile(
                                            [P, 2, L], MDT, tag="qkpair",
                                            name=f"qkp{c + 1}")
                                        gen = qk_gen(c + 1, wts_n, qkp_n)
                                    else:
                                        qkp_n = None
                                        gen = iter(())
                                    # prefetch wfc for the merged phase
                                    if c < 4:
                                        for j in (2 * c, 2 * c + 1):
                                            nc.sync.dma_start(
                                                wfc_sb[:, :,
                                                       j * 384:(j + 1) * 384],
                                                wfcv[:, :,
                                                     j * 384:(j + 1) * 384])
                                    PTs = [ptp.tile([P, LT, L], MDT,
                                                    tag=f"pt{par}",
                                                    name=f"pt{par}_{c}")
                                           for par in range(2)]
                                    for kt in range(4):
                                        emit_st(qkp_cur, PTs, kt)
                                        next(gen, None)
                                    psos0 = [ps_a.tile([P, 512], F32,
                                                       tag="pv",
                                                       name=f"pv0{par}")
                                             for par in range(2)]
                                    emit_pv(c, 0, PTs, psos0)
                                    emit_norm(c, 0, psos0)
                                    for kt in range(4, LT):
                                        emit_st(qkp_cur, PTs, kt)
                                        next(gen, None)
                                    psos1 = [ps_a.tile([P, 512], F32,
                                                       tag="pv",
                                                       name=f"pv1{par}")
                                             for par in range(2)]
                                    emit_pv(c, 1, PTs, psos1)
                                    for _ in gen:
                                        pass
                                    emit_norm(c, 1, psos1)
                                    qkp_cur = qkp_n

                        # ======== merged ph4 + ph5 + ph6 ========
                        z2T = fmp.tile([P, EC, L], MDT, tag="fm", name="z2T")
                        with (
                            tc.tile_pool(name="htp", bufs=1) as htp,
                            tc.tile_pool(name="wop", bufs=1) as wop,
                            tc.tile_pool(name="zp2", bufs=2) as zp2,
                            tc.tile_pool(name="stp2", bufs=3) as stp2,
                            tc.tile_pool(name="slp", bufs=2) as slp,
                            tc.tile_pool(name="osp", bufs=2) as osp,
                            tc.tile_pool(name="ps4", bufs=3,
                                         space="PSUM") as ps4,
                            tc.tile_pool(name="pstr", bufs=2,
                                         space="PSUM") as pstr,
                            tc.tile_pool(name="ps5", bufs=3,
                                         space="PSUM") as ps5,
                        ):
                            hT = htp.tile([P, KC2, L], MDT)
                            wo_a = wop.tile([P, KC2, 512], MDT)
                            nc.sync.dma_start(wo_a[:], woutv[:, :, 0:512])
                            wo_b = wop.tile([P, KC2, 256], MDT)
                            nc.sync.dma_start(wo_b[:], woutv[:, :, 512:768])

                            def proj_tile(t):
                                for (c0, cw) in ((0, 512), (512, 256)):
                                    pt = ps4.tile([P, 512], F32, tag="mm")
                                    for kc in range(EC):
                                        nc.tensor.matmul(
                                            pt[:, :cw],
                                            OT[:, kc, t * P:(t + 1) * P],
                                            wproj_sb[:, kc, c0:c0 + cw],
                                            start=(kc == 0),
                                            stop=(kc == EC - 1),
                                        )
                                    dst = x1bf[:, t, c0:c0 + cw]
                                    if gates["bproj"]:
                                        nc.vector.tensor_tensor(
                                            dst, pt[:, :cw],
                                            bproj_sb[:, c0:c0 + cw], OP.add)
                                        nc.vector.tensor_tensor(
                                            dst, dst, xbf[:, t, c0:c0 + cw],
                                            OP.add)
                                    else:
                                        nc.vector.tensor_tensor(
                                            dst, pt[:, :cw],
                                            xbf[:, t, c0:c0 + cw], OP.add)

                            def ln2_tile(t):
                                bnst = stp2.tile([P, 2, 6], F32, tag="bn2",
                                                 name=f"b2{t}")
                                xg = x1bf[:, t, :].rearrange(
                                    "p (n f) -> p n f", f=384)
                                for g in range(2):
                                    nc.vector.bn_stats(bnst[:, g, :],
                                                       xg[:, g, :])
                                mv = stp2.tile([P, 2], F32, tag="mv2",
                                               name=f"m2{t}")
                                nc.vector.bn_aggr(mv[:], bnst[:])
                                rt = magic_rsqrt(stp2, mv[:, 1:2], "2")
                                z2t = zp2.tile([P, E], MDT, tag="z2")
                                nc.vector.tensor_scalar(
                                    z2t[:], x1bf[:, t, :], mv[:, 0:1], rt[:],
                                    OP.subtract, OP.mult)
                                return z2t

                            def fc_groups():
                                for lcq in range(QC):
                                    for oc in range(KC2):
                                        pq = ps5.tile([P, 512], F32,
                                                      tag="fc")
                                        for kc in range(EC):
                                            nc.tensor.matmul(
                                                pq[:],
                                                wfc_sb[:, kc,
                                                       oc * P:(oc + 1) * P],
                                                z2T[:, kc,
                                                    lcq * 512:(lcq + 1) * 512],
                                                start=(kc == 0),
                                                stop=(kc == EC - 1),
                                            )
                                        pe_t = slp.tile([P, 512], F32,
                                                        tag="pe")
                                        bias = (bfce_sb[:, oc:oc + 1]
                                                if gates["bfc"] else lnla_b[:])
                                        nc.scalar.activation(
                                            pe_t[:], pq[:], AF.Exp, bias=bias,
                                            scale=1.0 / SELU_LAMBDA)
                                        rl_t = slp.tile([P, 512], F32,
                                                        tag="rl")
                                        if gates["bfc"]:
                                            nc.scalar.activation(
                                                rl_t[:], pq[:], AF.Relu,
                                                bias=bfcl_sb[:, oc:oc + 1])
                                        else:
                                            nc.scalar.activation(
                                                rl_t[:], pq[:], AF.Relu)
                                        # hT = relu + min(exp, la); the -la
                                        # constant is folded into bout
                                        nc.vector.scalar_tensor_tensor(
                                            hT[:, oc,
                                               lcq * 512:(lcq + 1) * 512],
                                            pe_t[:], SELU_LA, rl_t[:],
                                            OP.min, OP.add)
                                        yield

                            # tiles 0-3 transpose with a 1-tile lag (the
                            # LN2 chain hides under the next proj); fc lc=0
                            # starts right after TR(3), and TR(4..7) then
                            # slot between fc groups.
                            fcg = fc_groups()
                            zts = {}
                            for t in range(LT):
                                proj_tile(t)
                                zts[t] = ln2_tile(t)
                                if 1 <= t <= 4:
                                    transpose_block(z2T, zts.pop(t - 1),
                                                    t - 1, pstr)
                                if t >= 5:
                                    next(fcg, None)
                                    next(fcg, None)
                                    transpose_block(z2T, zts.pop(t - 1),
                                                    t - 1, pstr)
                            next(fcg, None)
                            next(fcg, None)
                            transpose_block(z2T, zts.pop(LT - 1), LT - 1,
                                            pstr)
                            for _ in fcg:
                                pass

                            # ---- ph6: out = h @ wout + x1 (two passes) ----
                            for t in range(LT):
                                pt = ps5.tile([P, 512], F32, tag="fc")
                                for kc in range(KC2):
                                    nc.tensor.matmul(
                                        pt[:], hT[:, kc, t * P:(t + 1) * P],
                                        wo_a[:, kc, :],
                                        start=(kc == 0), stop=(kc == KC2 - 1),
                                    )
                                ot = osp.tile([P, 512], F32, tag="ot")
                                if gates["bout"]:
                                    nc.vector.tensor_tensor(
                                        ot[:], pt[:], bout_sb[:, 0:512],
                                        OP.add)
                                    nc.vector.tensor_tensor(
                                        ot[:], ot[:], x1bf[:, t, 0:512],
                                        OP.add)
                                else:
                                    nc.vector.tensor_tensor(
                                        ot[:], pt[:], x1bf[:, t, 0:512],
                                        OP.add)
                                nc.sync.dma_start(outv[:, t, 0:512], ot[:])

                            for t in range(LT):
                                pt = ps5.tile([P, 512], F32, tag="fc")
                                for kc in range(KC2):
                                    nc.tensor.matmul(
                                        pt[:, :256],
                                        hT[:, kc, t * P:(t + 1) * P],
                                        wo_b[:, kc, :],
                                        start=(kc == 0), stop=(kc == KC2 - 1),
                                    )
                                ot = osp.tile([P, 512], F32, tag="ot")
                                if gates["bout"]:
                                    nc.vector.tensor_tensor(
                                        ot[:, :256], pt[:, :256],
                                        bout_sb[:, 512:768], OP.add)
                                    nc.vector.tensor_tensor(
                                        ot[:, :256], ot[:, :256],
                                        x1bf[:, t, 512:768], OP.add)
                                else:
                                    nc.vector.tensor_tensor(
                                        ot[:, :256], pt[:, :256],
                                        x1bf[:, t, 512:768], OP.add)
                                nc.sync.dma_start(outv[:, t, 512:768],
                                                  ot[:, :256])

    nc.finalize()
    return nc


def kernel(**inputs):
    global _last_results

    mm_dt_name = os.environ.get("KERNEL_MM_DT", "bf16")

    def arr(name):
        return np.ascontiguousarray(np.asarray(inputs[name], dtype=np.float32))

    x = arr("x")                       # [8, 1024, 768]
    g1 = arr("ln1_scale")
    b1 = arr("ln1_bias")
    w_qkv = arr("w_qkv")               # [768, 2304]
    b_qkv = arr("b_qkv")
    w_proj = arr("w_proj")
    b_proj = arr("b_proj")
    g2 = arr("ln2_scale")
    b2 = arr("ln2_bias")
    w_fc = arr("w_fc")
    b_fc = arr("b_fc")
    w_out = arr("w_out")
    b_out = arr("b_out")

    qscale = np.float32(1.0 / np.sqrt(D))

    w3 = w_qkv.reshape(E, H, 3, D)
    qw = (w3[:, :, 0, :].reshape(E, E) * qscale)
    kw = w3[:, :, 1, :].reshape(E, E)
    vw = w3[:, :, 2, :].reshape(E, E)
    wqk = np.ascontiguousarray(
        np.concatenate([qw, kw], axis=1) * g1[:, None]).astype(np.float32)
    wv = np.ascontiguousarray(vw * g1[:, None]).astype(np.float32)

    bq3 = (b1 @ w_qkv + b_qkv).reshape(H, 3, D)
    bqk = np.concatenate(
        [bq3[:, 0, :].reshape(E) * qscale, bq3[:, 1, :].reshape(E)]).astype(np.float32)
    bv = np.ascontiguousarray(bq3[:, 2, :].reshape(E)).astype(np.float32)

    wfc_p = np.ascontiguousarray(
        w_fc * g2[:, None] * np.float32(SELU_LAMBDA)).astype(np.float32)
    bfc_eff = (b2 @ w_fc + b_fc).astype(np.float32)
    bfce = (bfc_eff + np.float32(np.log(SELU_LA))).astype(np.float32)
    bfcl = (bfc_eff * np.float32(SELU_LAMBDA)).astype(np.float32)

    # hT carries selu + lambda*alpha; fold the constant back out via bout:
    # out -= la * colsum(wout)
    wdt = np.float32 if mm_dt_name == "f32r" else ml_dtypes.bfloat16
    wout_c = np.ascontiguousarray(w_out.astype(wdt))
    bout_eff = (b_out - np.float32(SELU_LA) *
                wout_c.astype(np.float32).sum(axis=0)).astype(np.float32)

    gates = {
        "bqk": bool(np.any(bqk != 0)),
        "bv": bool(np.any(bv != 0)),
        "bproj": bool(np.any(b_proj != 0)),
        "bfc": bool(np.any(bfc_eff != 0)),
        "bout": bool(np.any(bout_eff != 0)),
    }

    key = (tuple(sorted(gates.items())), mm_dt_name)
    if key not in _build_cache:
        _build_cache[key] = _build(gates, mm_dt_name)
    nc = _build_cache[key]

    def wcast(a):
        return np.ascontiguousarray(a.astype(wdt))

    base = {
        "wqk": wcast(wqk), "wv": wcast(wv),
        "wproj": wcast(w_proj),
        "wfc": wcast(wfc_p),
        "wout": wout_c,
    }
    if gates["bqk"]:
        base["bqk"] = bqk
    if gates["bv"]:
        base["bv"] = bv
    if gates["bproj"]:
        base["bproj"] = np.ascontiguousarray(b_proj)
    if gates["bfc"]:
        base["bfce"] = bfce
        base["bfcl"] = bfcl
    if gates["bout"]:
        base["bout"] = bout_eff

    in_maps = [dict(base, x=np.ascontiguousarray(x[c])) for c in range(NCORES)]
    res = bass_utils.run_bass_kernel_spmd(nc, in_maps, core_ids=list(range(NCORES)))
    _last_results = res
    out = np.stack([res.results[c]["out"] for c in range(NCORES)], axis=0)
    return out.astype(np.float32)
